# revision 1
# baseline (speedup 1.0000x reference)
"""Cross-attention kernel for Trainium2, distributed over 8 NeuronCores.

Sharding: data-parallel over batch (4) x tensor-parallel over head groups (2).
Core c handles batch b = c//2, heads [4g, 4g+4) with g = c%2.

Per-core device pipeline (layouts chosen so no on-device transposes are
needed; x^T / context^T are produced host-side as part of sharding):
  qT  = tanh(Wq_g^T @ x^T) * qmask          [256, 2048]   (d on partitions)
  kT  = tanh(Wk_g^T @ ctx^T), null col, pad [256, 2176]
  v   = ctx @ Wv_g (+ null row, ones col)   [2176, 4x65]  (j on partitions)
  S^T = exp(0.125 * kT_h^T qT_h + cmbias)   per (head, jtile, ichunk)
  outT_h = v_aug^T @ S^T  (row 64 = softmax denominator)
  rank-1 correction for masked queries, divide by denominator,
  out_partial = O @ Wo_g                    [2048, 512]
Host sums the two head-group partials per batch and adds bo.

PE instructions on TRN2 can carry at most ONE sync wait (walrus S3_LW /
ENGINE_NOP structs); Tile sometimes assigns more. `_split_pe_waits` runs
after scheduling and hoists extra waits onto PE nops inserted immediately
before the offending instruction — same engine stream, same blocking
semantics.
"""

import numpy as np

import concourse.bass as bass
import concourse.tile as tile
from concourse import bacc, bass_utils, mybir

FP = mybir.dt.float32
AF = mybir.ActivationFunctionType

B, N, M, DIM = 4, 2048, 2048, 512
HEADS, DH = 8, 64
G = 2          # head groups (tensor-parallel degree)
HG = 4         # heads per group
DG = HG * DH   # 256 dims per group
JT = 17        # j tiles of 128: 2048 context + null + 127 pad
JP = JT * 128  # 2176
NEG = -50.0    # additive mask bias (exp(-50) ~ 2e-22)
SCALE = 1.0 / np.sqrt(DH)  # 0.125
IC = 4         # i chunks of 512
VW = DH + 1    # v columns per head incl. ones column (den row)

LAST_RESULTS = None
_CACHE = {}


def _build():
    nc = bacc.Bacc("TRN2", debug=False, num_devices=8, enable_partition_id=False)
    d = {}

    def inp(name, shape):
        d[name] = nc.dram_tensor(name, shape, FP, kind="ExternalInput").ap()

    inp("xT", [DIM, N])
    inp("cxT", [DIM, M])
    inp("wq", [DIM, DG])
    inp("wk", [DIM, DG])
    inp("wv", [DIM, DG])
    inp("wo", [DG, DIM])
    inp("qm", [1, N])         # query mask as f32 row
    inp("cmf", [128, JT])     # context mask, padded+null, partition-major
    inp("nk", [128, 1])       # null_key tiled x2
    inp("nv", [1, HG * DH])   # null_value tiled x4
    d["out"] = nc.dram_tensor("out", [N, DIM], FP, kind="ExternalOutput").ap()

    with tile.TileContext(nc) as tc:
        _body(tc, d)
    nc.compile()
    return nc


_SPLIT_SKIP = (
    "InstDrain", "InstUnconditionalBranch", "InstCall",
    "InstEventSemaphore", "InstRegisterMove", "InstDmaTrigger",
)


def _split_pe_waits(nc):
    """Hoist all-but-one sync waits from compute-engine instructions onto
    fresh same-engine nops placed immediately before them (TRN2 TPB
    instruction structs accept only one sync wait in walrus codegen;
    drains/branches/DMA handle waits differently)."""
    engines = {
        mybir.EngineType.PE: nc.tensor,
        mybir.EngineType.Activation: nc.scalar,
        mybir.EngineType.DVE: nc.vector,
        mybir.EngineType.Pool: nc.gpsimd,
        mybir.EngineType.SP: nc.sync,
    }
    total = 0
    for bb in nc.m.functions[0].blocks:
        new_insts = []
        for ins in bb.instructions:
            si = ins.sync_info
            eng = engines.get(getattr(ins, "engine", None))
            if (
                eng is not None
                and type(ins).__name__ not in _SPLIT_SKIP
                and si is not None
                and si.on_wait
                and len(si.on_wait) > 1
            ):
                waits = list(si.on_wait)
                for w in waits[:-1]:
                    nop = eng._isa(
                        nc.isa.Opcode.NEURON_ISA_TPB_OPCODE_ENGINE_NOP,
                        {}, None, [], [], True,
                    )
                    nop.sync_info = mybir.SyncInfo(on_wait=[w], on_update=[])
                    nc.inst_map[nop.name] = nop
                    new_insts.append(nop)
                    total += 1
                si.on_wait = waits[-1:]
            new_insts.append(ins)
        bb.instructions = new_insts
    return total


def _body(tc, d):
    nc = tc.nc

    with (
        tc.tile_pool(name="consts", bufs=1) as consts,
        tc.tile_pool(name="big", bufs=1) as big,
        tc.tile_pool(name="spool", bufs=3) as spool,
        tc.tile_pool(name="small", bufs=2) as small,
        tc.tile_pool(name="mm", bufs=2, space="PSUM") as mm_ps,
        tc.tile_pool(name="acc", bufs=4, space="PSUM") as acc_ps,
        tc.tile_pool(name="rps", bufs=1, space="PSUM") as r_ps,
    ):
        # ---- constants / inputs ----
        wq = consts.tile([128, 4, DG], FP)
        nc.sync.dma_start(wq[:], d["wq"].rearrange("(c p) d -> p c d", p=128))
        wk = consts.tile([128, 4, DG], FP)
        nc.sync.dma_start(wk[:], d["wk"].rearrange("(c p) d -> p c d", p=128))
        wv = consts.tile([128, 4, DG], FP)
        nc.sync.dma_start(wv[:], d["wv"].rearrange("(c p) d -> p c d", p=128))
        wo = consts.tile([128, 2, DIM], FP)
        nc.sync.dma_start(wo[:], d["wo"].rearrange("(c p) o -> p c o", p=128))

        xT = big.tile([128, 4, N], FP)
        nc.sync.dma_start(xT[:], d["xT"].rearrange("(c p) i -> p c i", p=128))
        cxT = big.tile([128, 4, M], FP)
        nc.sync.dma_start(cxT[:], d["cxT"].rearrange("(c p) j -> p c j", p=128))

        qmB = big.tile([128, N], FP)  # query mask broadcast to 128 partitions
        nc.sync.dma_start(qmB[:], d["qm"].to_broadcast((128, N)))
        one_minus_qm = consts.tile([1, N], FP)
        nc.sync.dma_start(one_minus_qm[:], d["qm"])
        nc.scalar.activation(one_minus_qm[:], one_minus_qm[:], AF.Identity,
                             scale=-1.0, bias=1.0)

        cmf = consts.tile([128, JT], FP)
        nc.sync.dma_start(cmf[:], d["cmf"])
        negb = consts.tile([128, 1], FP)
        nc.vector.memset(negb[:], NEG)
        cmb = consts.tile([128, JT], FP)   # 0 where attendable, NEG where masked
        nc.scalar.activation(cmb[:], cmf[:], AF.Identity, scale=-NEG, bias=negb[:])
        cmexp = consts.tile([128, JT], FP)  # exp(cmb)
        nc.scalar.activation(cmexp[:], cmb[:], AF.Exp)
        negcm = consts.tile([128, JT], FP)  # -exp(cmb)
        nc.scalar.activation(negcm[:], cmexp[:], AF.Copy, scale=-1.0)

        nk = consts.tile([128, 1], FP)
        nc.sync.dma_start(nk[:], d["nk"])

        ones_col = consts.tile([128, 1], FP)
        nc.vector.memset(ones_col[:], 1.0)
        ones_pd = consts.tile([128, DH], FP)
        nc.vector.memset(ones_pd[:], 1.0)
        inv_row = consts.tile([1, 128], FP)
        nc.vector.memset(inv_row[:], 1.0 / (M + 1))

        qT = big.tile([128, 2, N], FP)
        kT = big.tile([128, 2, JP], FP)
        vsb = big.tile([128, JT, HG, VW], FP)
        Osb = big.tile([128, 2, N], FP)

        # ---- qT projection: qT[d, i] = tanh(sum_c Wq[c, d] x[i, c]) * qm[i]
        for dc in range(2):
            for ic in range(IC):
                ps = mm_ps.tile([128, 512], FP, tag="mm", name=f"psq{dc}{ic}")
                for cc in range(4):
                    nc.tensor.matmul(
                        ps[:],
                        wq[:, cc, dc * 128:(dc + 1) * 128],
                        xT[:, cc, ic * 512:(ic + 1) * 512],
                        start=(cc == 0), stop=(cc == 3),
                    )
                dst = qT[:, dc, ic * 512:(ic + 1) * 512]
                nc.scalar.activation(dst, ps[:], AF.Tanh)
                nc.vector.tensor_mul(dst, dst, qmB[:, ic * 512:(ic + 1) * 512])

        # ---- kT projection (+ tanh), null col, zero pad
        for dc in range(2):
            for jc in range(IC):
                ps = mm_ps.tile([128, 512], FP, tag="mm", name=f"psk{dc}{jc}")
                for cc in range(4):
                    nc.tensor.matmul(
                        ps[:],
                        wk[:, cc, dc * 128:(dc + 1) * 128],
                        cxT[:, cc, jc * 512:(jc + 1) * 512],
                        start=(cc == 0), stop=(cc == 3),
                    )
                nc.scalar.activation(kT[:, dc, jc * 512:(jc + 1) * 512], ps[:], AF.Tanh)
        nc.vector.memset(kT[:, :, M + 1:JP], 0.0)
        for dc in range(2):
            nc.scalar.activation(kT[:, dc, M:M + 1], nk[:], AF.Tanh)

        # ---- v projection: v[j, d]; last col of each head block = ones (denominator)
        nc.vector.memset(vsb[:, JT - 1, :, :], 0.0)
        for jt in range(JT - 1):
            ps = mm_ps.tile([128, DG], FP, tag="mm", name=f"psv{jt}")
            for cc in range(4):
                nc.tensor.matmul(
                    ps[:],
                    cxT[:, cc, jt * 128:(jt + 1) * 128],
                    wv[:, cc, :],
                    start=(cc == 0), stop=(cc == 3),
                )
            nc.vector.tensor_copy(
                vsb[:, jt, :, 0:DH],
                ps[:].rearrange("p (h e) -> p h e", h=HG),
            )
            nc.vector.memset(vsb[:, jt, :, DH:VW], 1.0)
        # null token row (j = M) lives at partition 0 of the last j tile
        nc.sync.dma_start(vsb[0:1, JT - 1, :, 0:DH],
                          d["nv"].rearrange("a (h e) -> a h e", h=HG))
        nc.vector.memset(vsb[0:1, JT - 1, :, DH:VW], 1.0)

        # ---- correction vectors (masked queries -> uniform attention)
        # corr_h = (scb/2049) * sum_all_j v_aug  -  sum_j exp(cmb_j) v_aug_j
        # (ones column of v_aug makes the denominator slot exactly 0)
        corr = consts.tile([1, HG, VW], FP)
        ps_scb = mm_ps.tile([1, JT], FP, tag="mm")
        nc.tensor.matmul(ps_scb[:], ones_col[:], cmexp[:], start=True, stop=True)
        scbrow = consts.tile([1, JT], FP)
        scb = consts.tile([1, 1], FP)
        nc.scalar.activation(scbrow[:], ps_scb[:], AF.Copy, accum_out=scb[:])
        ps_is = mm_ps.tile([128, 1], FP, tag="mm")
        nc.tensor.matmul(ps_is[:], inv_row[:], scb[:], start=True, stop=True)
        invscb = consts.tile([128, 1], FP)
        nc.scalar.copy(invscb[:], ps_is[:])
        for h in range(HG):
            ps_c = mm_ps.tile([1, VW], FP, tag="mm", name=f"psc{h}")
            for jt in range(JT):
                nc.tensor.matmul(ps_c[:], invscb[:], vsb[:, jt, h, :],
                                 start=(jt == 0), stop=False)
            for jt in range(JT):
                nc.tensor.matmul(ps_c[:], negcm[:, jt:jt + 1], vsb[:, jt, h, :],
                                 start=False, stop=(jt == JT - 1))
            nc.scalar.copy(corr[:, h, :], ps_c[:])

        # ---- flash attention over i chunks
        for ic in range(IC):
            isl = slice(ic * 512, (ic + 1) * 512)
            po = []
            for h in range(HG):
                po.append(acc_ps.tile([128, 512], FP, tag="po", name=f"po{ic}{h}"))
            for jt in range(JT):
                for h in range(HG):
                    pss = mm_ps.tile([128, 512], FP, tag="mm", name=f"pss{ic}{jt}{h}")
                    prow = 64 * (h % 2)
                    nc.tensor.matmul(
                        pss[:],
                        kT[prow:prow + DH, h // 2, jt * 128:(jt + 1) * 128],
                        qT[prow:prow + DH, h // 2, isl],
                        start=True, stop=True,
                    )
                    Ssb = spool.tile([128, 512], FP, tag="s", name=f"s{ic}{jt}{h}")
                    nc.scalar.activation(Ssb[:], pss[:], AF.Exp,
                                         bias=cmb[:, jt:jt + 1], scale=float(SCALE))
                    nc.tensor.matmul(
                        po[h][0:VW, :],
                        vsb[:, jt, h, :],
                        Ssb[:],
                        start=(jt == 0), stop=False,
                    )
            for h in range(HG):
                # rank-1 correction for masked queries (den row gets +0)
                nc.tensor.matmul(
                    po[h][0:VW, :],
                    corr[:, h, :],
                    one_minus_qm[:, isl],
                    start=False, stop=True,
                )
                den = small.tile([128, 512], FP, tag="den")
                nc.vector.tensor_copy(den[DH:VW, :], po[h][DH:VW, :])
                nc.vector.reciprocal(den[DH:VW, :], den[DH:VW, :])
                pr = r_ps.tile([DH, 512], FP, tag="pr", name=f"pr{ic}{h}")
                nc.tensor.matmul(pr[:], ones_pd[DH:VW, 0:DH], den[DH:VW, :],
                                 start=True, stop=True)
                prs = spool.tile([DH, 512], FP, tag="prs", name=f"prs{ic}{h}")
                nc.vector.tensor_copy(prs[:], pr[:])
                if h % 2 == 0:
                    nc.vector.tensor_mul(
                        Osb[0:DH, h // 2, isl], po[h][0:DH, :], prs[:])
                else:
                    ot = small.tile([DH, 512], FP, tag="ot")
                    nc.vector.tensor_mul(ot[:], po[h][0:DH, :], prs[:])
                    nc.sync.dma_start(Osb[DH:128, h // 2, isl], ot[:])

        # ---- output projection: out[i, o] = sum_hd O[hd, i] wo[hd, o]
        for it in range(N // 128):
            pf = mm_ps.tile([128, DIM], FP, tag="mm", name=f"pf{it}")
            for dc in range(2):
                nc.tensor.matmul(
                    pf[:],
                    Osb[:, dc, it * 128:(it + 1) * 128],
                    wo[:, dc, :],
                    start=(dc == 0), stop=(dc == 1),
                )
            fo = spool.tile([128, DIM], FP, tag="fo", name=f"fo{it}")
            nc.vector.tensor_copy(fo[:], pf[:])
            nc.sync.dma_start(d["out"][it * 128:(it + 1) * 128, :], fo[:])


def _core_inputs(inputs, core):
    b, g = core // 2, core % 2
    x = np.asarray(inputs["x"], np.float32)
    context = np.asarray(inputs["context"], np.float32)
    mask = np.asarray(inputs["mask"])
    context_mask = np.asarray(inputs["context_mask"])
    Wq = np.asarray(inputs["Wq"], np.float32)
    Wkv = np.asarray(inputs["Wkv"], np.float32)
    Wo = np.asarray(inputs["Wo"], np.float32)
    null_key = np.asarray(inputs["null_key"], np.float32)
    null_value = np.asarray(inputs["null_value"], np.float32)

    gs = slice(g * DG, (g + 1) * DG)
    cm = np.zeros(JP, np.float32)
    cm[:M] = context_mask[b].astype(np.float32)
    cm[M] = 1.0
    return {
        "xT": np.ascontiguousarray(x[b].T),
        "cxT": np.ascontiguousarray(context[b].T),
        "wq": np.ascontiguousarray(Wq[:, gs]),
        "wk": np.ascontiguousarray(Wkv[:, gs]),
        "wv": np.ascontiguousarray(Wkv[:, DIM + g * DG: DIM + (g + 1) * DG]),
        "wo": np.ascontiguousarray(Wo[gs, :]),
        "qm": mask[b].astype(np.float32).reshape(1, N),
        "cmf": np.ascontiguousarray(cm.reshape(JT, 128).T),
        "nk": np.ascontiguousarray(np.tile(null_key, 2).reshape(128, 1)),
        "nv": np.ascontiguousarray(np.tile(null_value, HG).reshape(1, HG * DH)),
    }


def kernel(x, context, mask, context_mask, Wq, Wkv, Wo, bo, null_key, null_value):
    global LAST_RESULTS
    inputs = {
        "x": x, "context": context, "mask": mask, "context_mask": context_mask,
        "Wq": Wq, "Wkv": Wkv, "Wo": Wo, "bo": bo,
        "null_key": null_key, "null_value": null_value,
    }
    if "nc" not in _CACHE:
        _CACHE["nc"] = _build()
    nc = _CACHE["nc"]
    in_maps = [_core_inputs(inputs, core) for core in range(8)]
    res = bass_utils.run_bass_kernel_spmd(nc, in_maps, core_ids=list(range(8)))
    LAST_RESULTS = res
    bo_np = np.asarray(bo, np.float32)
    out = np.empty((B, N, DIM), np.float32)
    for b in range(B):
        out[b] = res.results[2 * b]["out"] + res.results[2 * b + 1]["out"] + bo_np
    return out



# revision 10
# speedup vs baseline: 6.8733x; 6.8733x over previous
"""Cross-attention kernel for Trainium2, distributed over 8 NeuronCores.

Sharding: data-parallel over batch (4) x tensor-parallel over head groups (2).
Core c handles batch b = c//2, heads [4g, 4g+4) with g = c%2.

Per-core device pipeline (layouts chosen so no on-device transposes are
needed; x^T / context^T are produced host-side as part of sharding):
  qT  = tanh(Wq_g^T @ x^T) * qmask          [256, 2048]   (d on partitions)
  kT  = tanh(Wk_g^T @ ctx^T), null col, pad [256, 2176]
  v   = ctx @ Wv_g (+ null row, ones col)   [2176, 4x65]  (j on partitions)
  S^T = exp(0.125 * kT_h^T qT_h + cmbias)   per (head, jtile, ichunk)
  outT_h = v_aug^T @ S^T  (row 64 = softmax denominator)
  rank-1 correction for masked queries, divide by denominator,
  out_partial = O @ Wo_g                    [2048, 512]
Host sums the two head-group partials per batch and adds bo.

PE instructions on TRN2 can carry at most ONE sync wait (walrus S3_LW /
ENGINE_NOP structs); Tile sometimes assigns more. `_split_pe_waits` runs
after scheduling and hoists extra waits onto PE nops inserted immediately
before the offending instruction — same engine stream, same blocking
semantics.
"""

import ml_dtypes
import numpy as np

import concourse.bass as bass
import concourse.tile as tile
from concourse import bacc, bass_utils, mybir

FP = mybir.dt.float32
BF = mybir.dt.bfloat16
NPBF = ml_dtypes.bfloat16
AF = mybir.ActivationFunctionType

B, N, M, DIM = 4, 2048, 2048, 512
HEADS, DH = 8, 64
G = 2          # head groups (tensor-parallel degree)
HG = 4         # heads per group
DG = HG * DH   # 256 dims per group
JT = 17        # j tiles of 128: 2048 context + null + 127 pad
JP = JT * 128  # 2176
NEG = -50.0    # additive mask bias (exp(-50) ~ 2e-22)
SCALE = 1.0 / np.sqrt(DH)  # 0.125
IC = 4         # i chunks of 512
VW = DH + 1    # v columns per head incl. ones column (den row)

LAST_RESULTS = None
_CACHE = {}


def _build():
    nc = bacc.Bacc("TRN2", debug=False, num_devices=8, enable_partition_id=False)
    d = {}

    def inp(name, shape, dt=BF):
        d[name] = nc.dram_tensor(name, shape, dt, kind="ExternalInput").ap()

    inp("xT", [DIM, N])       # pre-masked host-side: x * mask
    inp("cxT", [DIM, M])
    inp("wq", [DIM, DG])
    inp("wk", [DIM, DG])
    inp("wv", [DIM, DG])
    inp("wo", [DG, DIM])
    inp("qm", [1, N])         # query mask as bf16 row
    inp("cmf", [128, JT], FP)  # context mask, padded+null, partition-major
    inp("nk", [128, 1])       # null_key tiled x2
    inp("nv", [1, HG * DH])   # null_value tiled x4
    d["out"] = nc.dram_tensor("out", [N, DIM], FP, kind="ExternalOutput").ap()

    with tile.TileContext(nc) as tc:
        _body(tc, d)
    nc.compile()
    return nc


_SPLIT_SKIP = (
    "InstDrain", "InstUnconditionalBranch", "InstCall",
    "InstEventSemaphore", "InstRegisterMove", "InstDmaTrigger",
)


def _split_pe_waits(nc):
    """Hoist all-but-one sync waits from compute-engine instructions onto
    fresh same-engine nops placed immediately before them (TRN2 TPB
    instruction structs accept only one sync wait in walrus codegen;
    drains/branches/DMA handle waits differently)."""
    engines = {
        mybir.EngineType.PE: nc.tensor,
        mybir.EngineType.Activation: nc.scalar,
        mybir.EngineType.DVE: nc.vector,
        mybir.EngineType.Pool: nc.gpsimd,
        mybir.EngineType.SP: nc.sync,
    }
    total = 0
    for bb in nc.m.functions[0].blocks:
        new_insts = []
        for ins in bb.instructions:
            si = ins.sync_info
            eng = engines.get(getattr(ins, "engine", None))
            if (
                eng is not None
                and type(ins).__name__ not in _SPLIT_SKIP
                and si is not None
                and si.on_wait
                and len(si.on_wait) > 1
            ):
                waits = list(si.on_wait)
                for w in waits[:-1]:
                    nop = eng._isa(
                        nc.isa.Opcode.NEURON_ISA_TPB_OPCODE_ENGINE_NOP,
                        {}, None, [], [], True,
                    )
                    nop.sync_info = mybir.SyncInfo(on_wait=[w], on_update=[])
                    nc.inst_map[nop.name] = nop
                    new_insts.append(nop)
                    total += 1
                si.on_wait = waits[-1:]
            new_insts.append(ins)
        bb.instructions = new_insts
    return total


def _body(tc, d):
    nc = tc.nc

    with (
        tc.tile_pool(name="consts", bufs=1) as consts,
        tc.tile_pool(name="big", bufs=1) as big,
        tc.tile_pool(name="spool", bufs=3) as spool,
        tc.tile_pool(name="small", bufs=2) as small,
        tc.tile_pool(name="mm", bufs=2, space="PSUM") as mm_ps,
        tc.tile_pool(name="acc", bufs=4, space="PSUM") as acc_ps,
        tc.tile_pool(name="rps", bufs=1, space="PSUM") as r_ps,
    ):
        # ---- constants / inputs ----
        wq = consts.tile([128, 4, DG], BF)
        nc.sync.dma_start(wq[:], d["wq"].rearrange("(c p) d -> p c d", p=128))
        wk = consts.tile([128, 4, DG], BF)
        nc.sync.dma_start(wk[:], d["wk"].rearrange("(c p) d -> p c d", p=128))
        wv = consts.tile([128, 4, DG], BF)
        nc.sync.dma_start(wv[:], d["wv"].rearrange("(c p) d -> p c d", p=128))
        wo = consts.tile([128, 2, DIM], BF)
        nc.sync.dma_start(wo[:], d["wo"].rearrange("(c p) o -> p c o", p=128))

        xT = big.tile([128, 4, N], BF)
        nc.sync.dma_start(xT[:], d["xT"].rearrange("(c p) i -> p c i", p=128))
        cxT = big.tile([128, 4, M], BF)
        nc.sync.dma_start(cxT[:], d["cxT"].rearrange("(c p) j -> p c j", p=128))

        one_minus_qm = consts.tile([1, N], BF)
        nc.sync.dma_start(one_minus_qm[:], d["qm"])
        nc.scalar.activation(one_minus_qm[:], one_minus_qm[:], AF.Identity,
                             scale=-1.0, bias=1.0)

        cmf = consts.tile([128, JT], FP)
        nc.sync.dma_start(cmf[:], d["cmf"])
        negb = consts.tile([128, 1], FP)
        nc.vector.memset(negb[:], NEG)
        cmb = consts.tile([128, JT], FP)   # 0 where attendable, NEG where masked
        nc.scalar.activation(cmb[:], cmf[:], AF.Identity, scale=-NEG, bias=negb[:])
        cmexp = consts.tile([128, JT], BF)  # exp(cmb)
        nc.scalar.activation(cmexp[:], cmb[:], AF.Exp)
        negcm = consts.tile([128, JT], BF)  # -exp(cmb)
        nc.scalar.activation(negcm[:], cmexp[:], AF.Copy, scale=-1.0)

        nk = consts.tile([128, 1], BF)
        nc.sync.dma_start(nk[:], d["nk"])

        ones_col = consts.tile([128, 1], BF)
        nc.vector.memset(ones_col[:], 1.0)
        ones_pd = consts.tile([128, DH], BF)
        nc.vector.memset(ones_pd[:], 1.0)
        inv_row = consts.tile([1, 128], FP)
        nc.vector.memset(inv_row[:], 1.0 / (M + 1))

        qT = big.tile([128, 2, N], BF)
        kT = big.tile([128, 2, JP], BF)
        vsb = big.tile([128, JT, HG, VW], BF)
        Osb = big.tile([128, 2, N], BF)

        # ---- qT projection: qT[d, i] = tanh(sum_c Wq[c, d] x[i, c])
        # (x is pre-masked host-side, so masked queries give q = 0)
        for dc in range(2):
            for ic in range(IC):
                ps = mm_ps.tile([128, 512], FP, tag="mm", name=f"psq{dc}{ic}")
                for cc in range(4):
                    nc.tensor.matmul(
                        ps[:],
                        wq[:, cc, dc * 128:(dc + 1) * 128],
                        xT[:, cc, ic * 512:(ic + 1) * 512],
                        start=(cc == 0), stop=(cc == 3),
                    )
                nc.scalar.activation(qT[:, dc, ic * 512:(ic + 1) * 512], ps[:],
                                     AF.Tanh)

        # ---- kT projection (+ tanh), null col, zero pad
        for dc in range(2):
            for jc in range(IC):
                ps = mm_ps.tile([128, 512], FP, tag="mm", name=f"psk{dc}{jc}")
                for cc in range(4):
                    nc.tensor.matmul(
                        ps[:],
                        wk[:, cc, dc * 128:(dc + 1) * 128],
                        cxT[:, cc, jc * 512:(jc + 1) * 512],
                        start=(cc == 0), stop=(cc == 3),
                    )
                nc.scalar.activation(kT[:, dc, jc * 512:(jc + 1) * 512], ps[:], AF.Tanh)
        nc.vector.memset(kT[:, :, M + 1:JP], 0.0)
        for dc in range(2):
            nc.scalar.activation(kT[:, dc, M:M + 1], nk[:], AF.Tanh)

        # ---- v projection: v[j, d]; last col of each head block = ones (denominator)
        nc.vector.memset(vsb[:, JT - 1, :, :], 0.0)
        for jt in range(JT - 1):
            ps = mm_ps.tile([128, DG], FP, tag="mm", name=f"psv{jt}")
            for cc in range(4):
                nc.tensor.matmul(
                    ps[:],
                    cxT[:, cc, jt * 128:(jt + 1) * 128],
                    wv[:, cc, :],
                    start=(cc == 0), stop=(cc == 3),
                )
            nc.vector.tensor_copy(
                vsb[:, jt, :, 0:DH],
                ps[:].rearrange("p (h e) -> p h e", h=HG),
            )
            nc.vector.memset(vsb[:, jt, :, DH:VW], 1.0)
        # null token row (j = M) lives at partition 0 of the last j tile
        nc.sync.dma_start(vsb[0:1, JT - 1, :, 0:DH],
                          d["nv"].rearrange("a (h e) -> a h e", h=HG))
        nc.vector.memset(vsb[0:1, JT - 1, :, DH:VW], 1.0)

        # ---- correction vectors (masked queries -> uniform attention)
        # corr_h = (scb/2049) * sum_all_j v_aug  -  sum_j exp(cmb_j) v_aug_j
        # (ones column of v_aug makes the denominator slot exactly 0)
        corr = consts.tile([1, HG, VW], BF)
        ps_scb = mm_ps.tile([1, JT], FP, tag="mm")
        nc.tensor.matmul(ps_scb[:], ones_col[:], cmexp[:], start=True, stop=True)
        scbrow = consts.tile([1, JT], FP)
        scb = consts.tile([1, 1], FP)
        nc.scalar.activation(scbrow[:], ps_scb[:], AF.Copy, accum_out=scb[:])
        ps_is = mm_ps.tile([128, 1], FP, tag="mm")
        nc.tensor.matmul(ps_is[:], inv_row[:], scb[:], start=True, stop=True)
        invscb = consts.tile([128, 1], BF)
        nc.scalar.copy(invscb[:], ps_is[:])
        for h in range(HG):
            ps_c = mm_ps.tile([1, VW], FP, tag="mm", name=f"psc{h}")
            for jt in range(JT):
                nc.tensor.matmul(ps_c[:], invscb[:], vsb[:, jt, h, :],
                                 start=(jt == 0), stop=False)
            for jt in range(JT):
                nc.tensor.matmul(ps_c[:], negcm[:, jt:jt + 1], vsb[:, jt, h, :],
                                 start=False, stop=(jt == JT - 1))
            nc.scalar.copy(corr[:, h, :], ps_c[:])

        # ---- flash attention over i chunks
        for ic in range(IC):
            isl = slice(ic * 512, (ic + 1) * 512)
            po = []
            for h in range(HG):
                po.append(acc_ps.tile([128, 512], FP, tag="po", name=f"po{ic}{h}"))
            for jt in range(JT):
                for h in range(HG):
                    pss = mm_ps.tile([128, 512], FP, tag="mm", name=f"pss{ic}{jt}{h}")
                    prow = 64 * (h % 2)
                    nc.tensor.matmul(
                        pss[:],
                        kT[prow:prow + DH, h // 2, jt * 128:(jt + 1) * 128],
                        qT[prow:prow + DH, h // 2, isl],
                        start=True, stop=True,
                    )
                    Ssb = spool.tile([128, 512], BF, tag="s", name=f"s{ic}{jt}{h}")
                    nc.scalar.activation(Ssb[:], pss[:], AF.Exp,
                                         bias=cmb[:, jt:jt + 1], scale=float(SCALE))
                    nc.tensor.matmul(
                        po[h][0:VW, :],
                        vsb[:, jt, h, :],
                        Ssb[:],
                        start=(jt == 0), stop=False,
                    )
            for h in range(HG):
                # rank-1 correction for masked queries (den row gets +0)
                nc.tensor.matmul(
                    po[h][0:VW, :],
                    corr[:, h, :],
                    one_minus_qm[:, isl],
                    start=False, stop=True,
                )
                den = small.tile([128, 512], BF, tag="den")
                nc.vector.tensor_copy(den[DH:VW, :], po[h][DH:VW, :])
                with nc.allow_low_precision(
                        reason="1/den in bf16; rel-err budget is 2e-2"):
                    nc.vector.reciprocal(den[DH:VW, :], den[DH:VW, :])
                pr = r_ps.tile([DH, 512], FP, tag="pr", name=f"pr{ic}{h}")
                nc.tensor.matmul(pr[:], ones_pd[DH:VW, 0:DH], den[DH:VW, :],
                                 start=True, stop=True)
                prs = spool.tile([DH, 512], BF, tag="prs", name=f"prs{ic}{h}")
                nc.vector.tensor_copy(prs[:], pr[:])
                if h % 2 == 0:
                    nc.vector.tensor_mul(
                        Osb[0:DH, h // 2, isl], po[h][0:DH, :], prs[:])
                else:
                    ot = small.tile([DH, 512], BF, tag="ot")
                    nc.vector.tensor_mul(ot[:], po[h][0:DH, :], prs[:])
                    nc.sync.dma_start(Osb[DH:128, h // 2, isl], ot[:])

        # ---- output projection: out[i, o] = sum_hd O[hd, i] wo[hd, o]
        for it in range(N // 128):
            pf = mm_ps.tile([128, DIM], FP, tag="mm", name=f"pf{it}")
            for dc in range(2):
                nc.tensor.matmul(
                    pf[:],
                    Osb[:, dc, it * 128:(it + 1) * 128],
                    wo[:, dc, :],
                    start=(dc == 0), stop=(dc == 1),
                )
            fo = spool.tile([128, DIM], FP, tag="fo", name=f"fo{it}")
            nc.vector.tensor_copy(fo[:], pf[:])
            nc.sync.dma_start(d["out"][it * 128:(it + 1) * 128, :], fo[:])


def _core_inputs(inputs, core):
    b, g = core // 2, core % 2
    x = np.asarray(inputs["x"], np.float32)
    context = np.asarray(inputs["context"], np.float32)
    mask = np.asarray(inputs["mask"])
    context_mask = np.asarray(inputs["context_mask"])
    Wq = np.asarray(inputs["Wq"], np.float32)
    Wkv = np.asarray(inputs["Wkv"], np.float32)
    Wo = np.asarray(inputs["Wo"], np.float32)
    null_key = np.asarray(inputs["null_key"], np.float32)
    null_value = np.asarray(inputs["null_value"], np.float32)

    gs = slice(g * DG, (g + 1) * DG)
    cm = np.zeros(JP, np.float32)
    cm[:M] = context_mask[b].astype(np.float32)
    cm[M] = 1.0
    xm = x[b] * mask[b].astype(np.float32)[:, None]  # masked queries -> q = 0
    return {
        "xT": np.ascontiguousarray(xm.T).astype(NPBF),
        "cxT": np.ascontiguousarray(context[b].T).astype(NPBF),
        "wq": np.ascontiguousarray(Wq[:, gs]).astype(NPBF),
        "wk": np.ascontiguousarray(Wkv[:, gs]).astype(NPBF),
        "wv": np.ascontiguousarray(
            Wkv[:, DIM + g * DG: DIM + (g + 1) * DG]).astype(NPBF),
        "wo": np.ascontiguousarray(Wo[gs, :]).astype(NPBF),
        "qm": mask[b].astype(NPBF).reshape(1, N),
        "cmf": np.ascontiguousarray(cm.reshape(JT, 128).T),
        "nk": np.tile(null_key, 2).reshape(128, 1).astype(NPBF),
        "nv": np.tile(null_value, HG).reshape(1, HG * DH).astype(NPBF),
    }


def kernel(x, context, mask, context_mask, Wq, Wkv, Wo, bo, null_key, null_value):
    global LAST_RESULTS
    inputs = {
        "x": x, "context": context, "mask": mask, "context_mask": context_mask,
        "Wq": Wq, "Wkv": Wkv, "Wo": Wo, "bo": bo,
        "null_key": null_key, "null_value": null_value,
    }
    if "nc" not in _CACHE:
        _CACHE["nc"] = _build()
    nc = _CACHE["nc"]
    in_maps = [_core_inputs(inputs, core) for core in range(8)]
    res = bass_utils.run_bass_kernel_spmd(nc, in_maps, core_ids=list(range(8)))
    LAST_RESULTS = res
    bo_np = np.asarray(bo, np.float32)
    out = np.empty((B, N, DIM), np.float32)
    for b in range(B):
        out[b] = res.results[2 * b]["out"] + res.results[2 * b + 1]["out"] + bo_np
    return out

